# revision 15
# baseline (speedup 1.0000x reference)
"""Trainium2 Bass kernel for nn_PolymerDistance (segment_reduce).

Problem: N = 2,097,152 atoms in M = 2048 molecules (1024 atoms each,
molecule_ix = arange(N)//1024 — sorted/contiguous).  The reference
centers both coordinate sets per molecule, forms the 3x3 cross
covariance, takes singular values (smallest sign-flipped by det),
and returns mean over molecules of var1 + var2 - 2*mean(sigma).

Strategy (hardcoded for the reference layout; verified at runtime with
a numpy fallback for anything else):
  - Shard atoms over 8 NeuronCores: 262,144 atoms = 256 whole molecules
    per core, so all segment reductions are core-local.
  - Per core, molecules map one-per-SBUF-partition in two groups of 128:
    tile [128, 3072] f32, each partition = one molecule's 1024 atoms x 3
    interleaved coords, contiguous in HBM => perfectly coalesced DMA.
  - Per-molecule sufficient statistics (raw moments):
      S12[i,j] = sum_a x2_i(a)*x1_j(a)   9 fused mul+reduce (VectorE
                                         tensor_tensor_reduce, strided)
      S1_j, S2_i                         ScalarE activation(Copy)+accum
      SS1, SS2 = sum-of-squares          ScalarE activation(Square)+accum
    cov = S12/K - m2 m1^T reconstructed algebraically afterwards.
  - Tiny per-molecule tail fully on device: C = cov^T cov, eigenvalues
    via Newton on the traceless-normalized characteristic cubic
    (8u^3+r)/(12u^2-3) — no acos/cos tables; sigma = sqrt(eig);
    smallest sigma sign-flipped by sign(det cov);
    dist = var1 + var2 - (2/3)(s_max + s_mid + sgn*s_min).
  - Host glue: slice per-core inputs, SPMD over cores 0-7, mean of the
    2048 per-molecule distances.
"""

import numpy as np

N = 2_097_152
M = 2048
K = 1024          # atoms per molecule
NCORES = 8
NC_ATOMS = N // NCORES        # 262144
NC_MOLS = M // NCORES         # 256
G = 2                         # molecule groups of 128 per core
P = 128
W = 3 * K                     # 3072 free f32 per molecule
INVK = 1.0 / K

_CACHE = {}


# ----------------------------------------------------------------------
# Bass program
# ----------------------------------------------------------------------

def _sv(base_ap, off, dims):
    """Strided free-dim view of a 2D SBUF tile AP.

    dims = [(step, count), ...] in elements; partition dim kept as-is.
    """
    import concourse.bass as bass

    ap0 = base_ap.ap[0]
    return bass.AP(
        tensor=base_ap.tensor,
        offset=base_ap.offset + off,
        ap=[[ap0[0], ap0[1]]] + [[s, c] for (s, c) in dims],
    )


def _build_kernel_body(tc, c1_ap, c2_ap, dist_ap):
    import concourse.mybir as mybir

    nc = tc.nc
    f32 = mybir.dt.float32
    mult = mybir.AluOpType.mult
    add = mybir.AluOpType.add
    sub = mybir.AluOpType.subtract
    AX = mybir.AxisListType.X
    AF = mybir.ActivationFunctionType

    # DRAM views: [NC_ATOMS, 3] -> [G, 128, 3072]
    c1v = c1_ap.rearrange("(g p a) c -> g p (a c)", g=G, p=P)
    c2v = c2_ap.rearrange("(g p a) c -> g p (a c)", g=G, p=P)

    import contextlib

    with contextlib.ExitStack() as ctx:
        cpool = ctx.enter_context(tc.tile_pool(name="coords", bufs=2))
        spool = ctx.enter_context(tc.tile_pool(name="stats", bufs=1))
        wpool = ctx.enter_context(tc.tile_pool(name="work", bufs=1))
        dpool = ctx.enter_context(tc.tile_pool(name="dummies", bufs=2))
        ppool = ctx.enter_context(tc.tile_pool(name="prod", bufs=3))

        # stats layout: 17 stats x G groups, col = 2*k + g
        #  k 0..8  : S12[i,j] at k=3i+j
        #  k 9..11 : S1_j     (coords1 sums)
        #  k12..14 : S2_i     (coords2 sums)
        #  k15     : SS1, k16: SS2
        ST = spool.tile([P, 34], f32)
        STa = ST[:, :]

        def stcol(k, g):
            return STa[:, 2 * k + g : 2 * k + g + 1]

        # ---------------- per-group statistics ----------------
        # Engine balance: DVE carries 13 fused STT dot products + the tail;
        # ScalarE carries SS/S accumulation + 5 dense product-plane reduces.
        ACT_PAIRS = {(2, 0), (2, 1), (2, 2), (1, 2), (0, 2)}
        first_compute_inst = None
        g1_dma_insts = []
        for g in range(G):
            V1 = cpool.tile([P, W], f32, tag="v1")
            V2 = cpool.tile([P, W], f32, tag="v2")
            d1 = nc.sync.dma_start(out=V1, in_=c1v[g])
            d2 = nc.sync.dma_start(out=V2, in_=c2v[g])
            if g == 1:
                g1_dma_insts = [d1, d2]
            V1a = V1[:, :]
            V2a = V2[:, :]

            dummy_v = dpool.tile([P, 1], f32, tag="dv")
            dummy_s = dpool.tile([P, 1], f32, tag="ds")

            def plane(Va, d):
                return _sv(Va, d, [(3, K)])

            # S12: fused multiply + free-dim reduce on VectorE
            # (scalar_tensor_tensor: out = (in0*1.0)*in1, accum = sum(out);
            #  tensor_tensor_reduce is a custom DVE op that faults on this
            #  runtime, STT is standard ISA and does the same fused job).
            # ACT_PAIRS instead: strided TT product -> dense plane, reduced by
            # a dense ScalarE Copy+accum (cheaper than strided ACT S-copies).
            for i in range(3):
                for j in range(3):
                    if (i, j) in ACT_PAIRS:
                        PRD = ppool.tile([P, K], f32, tag="prd")
                        nc.vector.tensor_tensor(
                            out=PRD[:, :],
                            in0=plane(V2a, i),
                            in1=plane(V1a, j),
                            op=mult,
                        )
                        nc.scalar.activation(
                            out=dummy_s[:, :].broadcast_to((P, K)),
                            in_=PRD[:, :],
                            func=AF.Copy,
                            accum_out=stcol(3 * i + j, g),
                        )
                    else:
                        inst = nc.vector.scalar_tensor_tensor(
                            out=dummy_v[:, :].broadcast_to((P, K)),
                            in0=plane(V2a, i),
                            scalar=1.0,
                            in1=plane(V1a, j),
                            op0=mult,
                            op1=mult,
                            accum_out=stcol(3 * i + j, g),
                        )
                        if first_compute_inst is None:
                            first_compute_inst = inst

            # SS on ScalarE: Square + accumulate over the whole 3072
            nc.scalar.activation(
                out=dummy_s[:, :].broadcast_to((P, W)),
                in_=V1a,
                func=AF.Square,
                accum_out=stcol(15, g),
            )
            nc.scalar.activation(
                out=dummy_s[:, :].broadcast_to((P, W)),
                in_=V2a,
                func=AF.Square,
                accum_out=stcol(16, g),
            )
            # S (per-coord sums) on ScalarE: Copy + accumulate, strided
            for d in range(3):
                nc.scalar.activation(
                    out=dummy_s[:, :].broadcast_to((P, K)),
                    in_=plane(V1a, d),
                    func=AF.Copy,
                    accum_out=stcol(9 + d, g),
                )
                nc.scalar.activation(
                    out=dummy_s[:, :].broadcast_to((P, K)),
                    in_=plane(V2a, d),
                    func=AF.Copy,
                    accum_out=stcol(12 + d, g),
                )

        # Group-1 loads wait for group-0 compute to start so the SDMA engines
        # give group 0 the full HBM bandwidth first (earlier compute start).
        from concourse.tile_rust import add_dep_helper

        for dma_inst in g1_dma_insts:
            add_dep_helper(
                dma_inst.ins, first_compute_inst.ins, sync=True,
                reason="serialize g1 loads behind g0 compute start",
            )

        # ---------------- per-molecule tail ----------------
        def wt(name, w):
            t = wpool.tile([P, w], f32, tag=name)
            return t[:, :]

        T1 = wt("t1", 18)
        COV = wt("cov", 18)
        CCT = wt("cct", 54)
        C18 = wt("c18", 18)
        CS = wt("cs", 18)
        Q = wt("q", 2)
        P2 = wt("p2", 2)
        RP = wt("rp", 2)
        RPW = wt("rpw", 2)
        DT = wt("dt", 36)
        U = wt("u", 36)
        MI = wt("mi", 12)
        W12 = wt("w12", 12)
        DET4 = wt("det4", 4)
        R2 = wt("r2", 2)
        MU = wt("mu", 4)
        MU2 = wt("mu2", 4)
        MU3 = wt("mu3", 4)
        NUM = wt("num", 4)
        DEN = wt("den", 4)
        RD = wt("rd", 4)
        LV4 = wt("lv4", 4)
        MID2 = wt("mid2", 2)
        SG = wt("sg", 2)
        S3 = wt("s3", 2)
        SQ12 = wt("sq12", 12)
        SSUM4 = wt("ssum4", 4)
        V4 = wt("v4", 4)
        VS2 = wt("vs2", 2)
        DIST2 = wt("dist2", 2)

        tsc = nc.vector.tensor_scalar
        stt = nc.vector.scalar_tensor_tensor
        tt = nc.vector.tensor_tensor

        # outer[i,j,g] = S2_i * S1_j
        tt(
            out=_sv(T1, 0, [(6, 3), (2, 3), (1, 2)]),
            in0=_sv(STa, 24, [(2, 3), (0, 3), (1, 2)]),
            in1=_sv(STa, 18, [(0, 3), (2, 3), (1, 2)]),
            op=mult,
        )
        # cov = (S12 - outer*invK) * invK     (per-atom cross covariance)
        tsc(out=T1, in0=T1, scalar1=INVK, scalar2=None, op0=mult)
        nc.vector.tensor_sub(COV, _sv(STa, 0, [(1, 18)]), T1)
        tsc(out=COV, in0=COV, scalar1=INVK, scalar2=None, op0=mult)

        # C = cov^T cov (Gram, symmetric PSD): CCT[a,b,g,i] = cov[i,a,g]*cov[i,b,g]
        # (DVE APs max 3 free dims -> split over a)
        for a in range(3):
            tt(
                out=_sv(CCT, 18 * a, [(6, 3), (3, 2), (1, 3)]),
                in0=_sv(COV, 2 * a, [(0, 3), (1, 2), (6, 3)]),
                in1=_sv(COV, 0, [(2, 3), (1, 2), (6, 3)]),
                op=mult,
            )
        nc.vector.reduce_sum(
            out=_sv(C18, 0, [(2, 9), (1, 2)]),
            in_=_sv(CCT, 0, [(6, 9), (3, 2), (1, 3)]),
            axis=AX,
        )
        # q = tr(C)/3
        nc.vector.reduce_sum(out=Q, in_=_sv(C18, 0, [(1, 2), (8, 3)]), axis=AX)
        tsc(out=Q, in0=Q, scalar1=1.0 / 3.0, scalar2=None, op0=mult)
        # C18 := Cq = C - q I (traceless); diag d_a = C_aa - q
        nc.vector.tensor_sub(
            _sv(C18, 0, [(1, 2), (8, 3)]),
            _sv(C18, 0, [(1, 2), (8, 3)]),
            _sv(Q, 0, [(1, 2), (0, 3)]),
        )
        # p2 = sum_ab Cq_ab^2 ; p = sqrt(p2/6 + eps); rp = 1/p
        tt(out=CS, in0=C18, in1=C18, op=mult)
        nc.vector.reduce_sum(out=P2, in_=_sv(CS, 0, [(1, 2), (2, 9)]), axis=AX)
        tsc(out=P2, in0=P2, scalar1=1.0 / 6.0, scalar2=1e-12, op0=mult, op1=add)
        nc.scalar.activation(out=P2, in_=P2, func=AF.Sqrt)
        nc.vector.reciprocal(out=RP, in_=P2)

        # DT[a,b,m,g]: m=0 -> cov, m=1 -> Cq; batched 3x3 determinant
        nc.vector.tensor_copy(
            _sv(DT, 0, [(12, 3), (4, 3), (1, 2)]), _sv(COV, 0, [(6, 3), (2, 3), (1, 2)])
        )
        nc.vector.tensor_copy(
            _sv(DT, 2, [(12, 3), (4, 3), (1, 2)]), _sv(C18, 0, [(6, 3), (2, 3), (1, 2)])
        )
        # u[x,y,m,g] = DT[1,x]*DT[2,y]  (split over x: DVE APs max 3 free dims)
        for x in range(3):
            tt(
                out=_sv(U, 12 * x, [(4, 3), (2, 2), (1, 2)]),
                in0=_sv(DT, 12 + 4 * x, [(0, 3), (2, 2), (1, 2)]),
                in1=_sv(DT, 24, [(4, 3), (2, 2), (1, 2)]),
                op=mult,
            )
        # minors
        nc.vector.tensor_sub(
            _sv(MI, 0, [(2, 2), (1, 2)]), _sv(U, 20, [(2, 2), (1, 2)]), _sv(U, 28, [(2, 2), (1, 2)])
        )
        nc.vector.tensor_sub(
            _sv(MI, 4, [(2, 2), (1, 2)]), _sv(U, 8, [(2, 2), (1, 2)]), _sv(U, 24, [(2, 2), (1, 2)])
        )
        nc.vector.tensor_sub(
            _sv(MI, 8, [(2, 2), (1, 2)]), _sv(U, 4, [(2, 2), (1, 2)]), _sv(U, 12, [(2, 2), (1, 2)])
        )
        # det = c00*M0 - c01*M1 + c02*M2
        tt(
            out=W12,
            in0=_sv(DT, 0, [(4, 3), (2, 2), (1, 2)]),
            in1=_sv(MI, 0, [(4, 3), (2, 2), (1, 2)]),
            op=mult,
        )
        nc.vector.tensor_sub(DET4, _sv(W12, 0, [(2, 2), (1, 2)]), _sv(W12, 4, [(2, 2), (1, 2)]))
        nc.vector.tensor_add(DET4, DET4, _sv(W12, 8, [(2, 2), (1, 2)]))

        # r = clamp(det(Cq)/(2 p^3), [-1, 1])
        nc.vector.tensor_mul(RPW, RP, RP)
        nc.vector.tensor_mul(RPW, RPW, RP)
        nc.vector.tensor_mul(R2, _sv(DET4, 2, [(1, 2)]), RPW)
        tsc(out=R2, in0=R2, scalar1=0.5, scalar2=1.0, op0=mult, op1=mybir.AluOpType.min)
        tsc(out=R2, in0=R2, scalar1=-1.0, scalar2=None, op0=mybir.AluOpType.max)

        # Solve 4u^3 - 3u = r (roots are cos(acos(r)/3 + 2pi k/3)).
        # Fold to a = |r| in [0, 1]: the largest root u1(a) in [0.866, 1] is
        # always well-separated (gap >= 0.37), so Newton converges fast from a
        # quadratic init; the other two roots come exactly from quadratic
        # deflation u = (-u1 +- sqrt(3 - 3 u1^2))/2 — exact at double roots,
        # where plain Newton is only linearly convergent.
        SGR = wt("sgr", 2)    # sign(r)
        AR = wt("ar", 2)      # |r|
        MUA = wt("mua", 2)    # Newton iterate (largest root for a)
        MSQ = wt("msq", 2)
        DSC = wt("dsc", 2)
        MU6 = wt("mu6", 6)    # (u_max, u_mid, u_min) x g for the original r
        tsc(out=SGR, in0=R2, scalar1=0.0, scalar2=None, op0=mybir.AluOpType.is_lt)
        tsc(out=SGR, in0=SGR, scalar1=-2.0, scalar2=1.0, op0=mult, op1=add)
        nc.vector.tensor_mul(AR, R2, SGR)
        # init: fit of cos(acos(a)/3) at a in {0, .5, 1}; |err| < 3e-3
        nc.vector.tensor_mul(MSQ, AR, AR)
        tsc(out=MSQ, in0=MSQ, scalar1=-0.0268, scalar2=0.8660, op0=mult, op1=add)
        stt(out=MUA, in0=AR, scalar=0.1608, in1=MSQ, op0=mult, op1=add)
        MCU = wt("mcu", 2)
        NU2 = wt("nu2", 2)
        DE2 = wt("de2", 2)
        RD2 = wt("rd2", 2)
        for _ in range(3):
            nc.vector.tensor_mul(MSQ, MUA, MUA)
            nc.vector.tensor_mul(MCU, MSQ, MUA)
            stt(out=NU2, in0=MCU, scalar=8.0, in1=AR, op0=mult, op1=add)
            tsc(out=DE2, in0=MSQ, scalar1=12.0, scalar2=-3.0, op0=mult, op1=add)
            nc.vector.reciprocal(out=RD2, in_=DE2)
            nc.vector.tensor_mul(MUA, NU2, RD2)
        # deflation: disc = sqrt(max(3 - 3 u1^2, 0)); u2 = (disc - u1)/2 (mid),
        # u3 = -(u1 + disc)/2 (smallest)
        nc.vector.tensor_mul(MSQ, MUA, MUA)
        tsc(out=MSQ, in0=MSQ, scalar1=-3.0, scalar2=3.0, op0=mult, op1=add)
        tsc(out=MSQ, in0=MSQ, scalar1=0.0, scalar2=None, op0=mybir.AluOpType.max)
        nc.scalar.activation(out=DSC, in_=MSQ, func=AF.Sqrt)
        # u_max(r) = sgn * (r>=0 ? u1 : u3);  u_min(r) = sgn * (r>=0 ? u3 : u1)
        # with m = (sgn+1)/2: u_max = sgn*(u3 + m*(u1-u3)), u_min = sgn*(u1 - m*(u1-u3))
        MM = wt("mm", 2)      # m
        U3 = wt("u3", 2)
        D13 = wt("d13", 2)
        tsc(out=MM, in0=SGR, scalar1=1.0, scalar2=0.5, op0=add, op1=mult)
        nc.vector.tensor_add(U3, MUA, DSC)
        tsc(out=U3, in0=U3, scalar1=-0.5, scalar2=None, op0=mult)
        nc.vector.tensor_sub(D13, MUA, U3)
        # u_mid(r) = sgn * (disc - u1)/2
        nc.vector.tensor_sub(_sv(MU6, 2, [(1, 2)]), DSC, MUA)
        tsc(out=_sv(MU6, 2, [(1, 2)]), in0=_sv(MU6, 2, [(1, 2)]), scalar1=0.5,
            scalar2=None, op0=mult)
        nc.vector.tensor_mul(_sv(MU6, 2, [(1, 2)]), _sv(MU6, 2, [(1, 2)]), SGR)
        MD = wt("md", 2)
        nc.vector.tensor_mul(MD, MM, D13)
        nc.vector.tensor_add(_sv(MU6, 0, [(1, 2)]), U3, MD)
        nc.vector.tensor_mul(_sv(MU6, 0, [(1, 2)]), _sv(MU6, 0, [(1, 2)]), SGR)
        nc.vector.tensor_sub(_sv(MU6, 4, [(1, 2)]), MUA, MD)
        nc.vector.tensor_mul(_sv(MU6, 4, [(1, 2)]), _sv(MU6, 4, [(1, 2)]), SGR)

        # lambda_k = q + 2 p u_k; sigma = sqrt(lambda); LS6 = (max, mid, min) x g
        LS6 = wt("ls6", 6)
        nc.vector.tensor_mul(LS6, MU6, _sv(P2, 0, [(0, 3), (1, 2)]))
        tsc(out=LS6, in0=LS6, scalar1=2.0, scalar2=None, op0=mult)
        nc.vector.tensor_add(LS6, LS6, _sv(Q, 0, [(0, 3), (1, 2)]))
        tsc(out=LS6, in0=LS6, scalar1=0.0, scalar2=None, op0=mybir.AluOpType.max)
        nc.scalar.activation(out=LS6, in_=LS6, func=AF.Sqrt)

        # sgn = sign(det cov) applied to smallest sigma
        tsc(out=SG, in0=_sv(DET4, 0, [(1, 2)]), scalar1=0.0, scalar2=None,
            op0=mybir.AluOpType.is_lt)
        tsc(out=SG, in0=SG, scalar1=-2.0, scalar2=1.0, op0=mult, op1=add)
        nc.vector.tensor_mul(SG, SG, _sv(LS6, 4, [(1, 2)]))
        nc.vector.tensor_add(S3, _sv(LS6, 0, [(1, 2)]), _sv(LS6, 2, [(1, 2)]))
        nc.vector.tensor_add(S3, S3, SG)

        # var_t = (SS_t - |S_t|^2 * invK) * invK / 3, t in {1, 2}
        nc.vector.tensor_mul(SQ12, _sv(STa, 18, [(1, 12)]), _sv(STa, 18, [(1, 12)]))
        nc.vector.reduce_sum(
            out=SSUM4, in_=_sv(SQ12, 0, [(6, 2), (1, 2), (2, 3)]), axis=AX
        )
        stt(out=V4, in0=SSUM4, scalar=-INVK, in1=_sv(STa, 30, [(1, 4)]), op0=mult, op1=add)
        tsc(out=V4, in0=V4, scalar1=INVK / 3.0, scalar2=None, op0=mult)
        nc.vector.tensor_add(VS2, _sv(V4, 0, [(1, 2)]), _sv(V4, 2, [(1, 2)]))

        # dist = var1 + var2 - (2/3)(s_max + s_mid + sgn*s_min)
        tsc(out=S3, in0=S3, scalar1=-2.0 / 3.0, scalar2=None, op0=mult)
        nc.vector.tensor_add(DIST2, VS2, S3)

        nc.sync.dma_start(out=dist_ap, in_=DIST2)


def _build():
    if "nc" in _CACHE:
        return _CACHE["nc"]
    import concourse.bacc as bacc
    import concourse.tile as tile
    import concourse.mybir as mybir

    nc = bacc.Bacc("TRN2", target_bir_lowering=False, debug=False)
    c1 = nc.dram_tensor("coords1", [NC_ATOMS, 3], mybir.dt.float32, kind="ExternalInput")
    c2 = nc.dram_tensor("coords2", [NC_ATOMS, 3], mybir.dt.float32, kind="ExternalInput")
    dist = nc.dram_tensor("dist", [P, G], mybir.dt.float32, kind="ExternalOutput")
    with tile.TileContext(nc) as tc:
        _build_kernel_body(tc, c1.ap(), c2.ap(), dist.ap())
    nc.compile()
    _CACHE["nc"] = nc
    return nc


# ----------------------------------------------------------------------
# Host glue
# ----------------------------------------------------------------------

def _expected_molecule_ix():
    return (np.arange(N, dtype=np.int64) // K).astype(np.int32)


def _numpy_fallback(coords1, coords2, molecule_ix):
    """Exact mirror of the reference for unexpected input layouts."""
    c1 = np.asarray(coords1, np.float64)
    c2 = np.asarray(coords2, np.float64)
    mol = np.asarray(molecule_ix, np.int64)
    m = M
    counts = np.bincount(mol, minlength=m).astype(np.float64)

    def seg_sum(x):
        out = np.zeros((m,) + x.shape[1:], np.float64)
        np.add.at(out, mol, x)
        return out

    cnt = counts.reshape((m,) + (1,) * 1)
    m1 = seg_sum(c1) / cnt
    m2 = seg_sum(c2) / cnt
    d1 = c1 - m1[mol]
    d2 = c2 - m2[mol]
    outer = d1[:, None, :] * d2[:, :, None]
    cov = seg_sum(outer.reshape(-1, 9)).reshape(m, 3, 3) / counts[:, None, None]
    sigma = np.linalg.svd(cov, compute_uv=False)
    det = np.linalg.det(cov)
    sigma[det < 0, 2] *= -1.0
    sig = sigma.mean(-1)
    var1 = (seg_sum(d1**2) / counts[:, None]).mean(-1)
    var2 = (seg_sum(d2**2) / counts[:, None]).mean(-1)
    dist = var1 + var2 - 2.0 * sig
    return np.float32(dist.mean())


def kernel(coords1, coords2, molecule_ix, _want_results=False):
    coords1 = np.ascontiguousarray(np.asarray(coords1, np.float32))
    coords2 = np.ascontiguousarray(np.asarray(coords2, np.float32))
    molecule_ix = np.asarray(molecule_ix)

    if (
        coords1.shape != (N, 3)
        or molecule_ix.shape != (N,)
        or not np.array_equal(molecule_ix.astype(np.int64), _expected_molecule_ix())
    ):
        return _numpy_fallback(coords1, coords2, molecule_ix)

    from concourse import bass_utils

    nc = _build()
    in_maps = [
        {
            "coords1": np.ascontiguousarray(coords1[c * NC_ATOMS : (c + 1) * NC_ATOMS]),
            "coords2": np.ascontiguousarray(coords2[c * NC_ATOMS : (c + 1) * NC_ATOMS]),
        }
        for c in range(NCORES)
    ]
    res = bass_utils.run_bass_kernel_spmd(nc, in_maps, core_ids=list(range(NCORES)))
    # dist[p, g] = molecule (g*128 + p) of that core's 256
    per_mol = np.concatenate(
        [np.asarray(r["dist"]).T.reshape(-1) for r in res.results]
    )
    out = np.float32(per_mol.mean(dtype=np.float64))
    if _want_results:
        return out, res, per_mol
    return out


# revision 16
# speedup vs baseline: 1.3315x; 1.3315x over previous
"""Trainium2 Bass kernel for nn_PolymerDistance (segment_reduce).

Problem: N = 2,097,152 atoms in M = 2048 molecules (1024 atoms each,
molecule_ix = arange(N)//1024 — sorted/contiguous).  The reference
centers both coordinate sets per molecule, forms the 3x3 cross
covariance, takes singular values (smallest sign-flipped by det),
and returns mean over molecules of var1 + var2 - 2*mean(sigma).

Strategy (hardcoded for the reference layout; verified at runtime with
a numpy fallback for anything else):
  - Shard atoms over 8 NeuronCores: 262,144 atoms = 256 whole molecules
    per core, so all segment reductions are core-local.
  - Per core, molecules map one-per-SBUF-partition in two groups of 128:
    tile [128, 3072] f32, each partition = one molecule's 1024 atoms x 3
    interleaved coords, contiguous in HBM => perfectly coalesced DMA.
  - Per-molecule sufficient statistics (raw moments):
      S12[i,j] = sum_a x2_i(a)*x1_j(a)   9 fused mul+reduce (VectorE
                                         tensor_tensor_reduce, strided)
      S1_j, S2_i                         ScalarE activation(Copy)+accum
      SS1, SS2 = sum-of-squares          ScalarE activation(Square)+accum
    cov = S12/K - m2 m1^T reconstructed algebraically afterwards.
  - Tiny per-molecule tail fully on device: C = cov^T cov, eigenvalues
    via Newton on the traceless-normalized characteristic cubic
    (8u^3+r)/(12u^2-3) — no acos/cos tables; sigma = sqrt(eig);
    smallest sigma sign-flipped by sign(det cov);
    dist = var1 + var2 - (2/3)(s_max + s_mid + sgn*s_min).
  - Host glue: slice per-core inputs, SPMD over cores 0-7, mean of the
    2048 per-molecule distances.
"""

import numpy as np

N = 2_097_152
M = 2048
K = 1024          # atoms per molecule
NCORES = 8
NC_ATOMS = N // NCORES        # 262144
NC_MOLS = M // NCORES         # 256
G = 2                         # molecule groups of 128 per core
P = 128
W = 3 * K                     # 3072 free f32 per molecule
INVK = 1.0 / K

_CACHE = {}


# ----------------------------------------------------------------------
# Bass program
# ----------------------------------------------------------------------

def _sv(base_ap, off, dims):
    """Strided free-dim view of a 2D SBUF tile AP.

    dims = [(step, count), ...] in elements; partition dim kept as-is.
    """
    import concourse.bass as bass

    ap0 = base_ap.ap[0]
    return bass.AP(
        tensor=base_ap.tensor,
        offset=base_ap.offset + off,
        ap=[[ap0[0], ap0[1]]] + [[s, c] for (s, c) in dims],
    )


def _build_kernel_body(tc, c1_ap, c2_ap, dist_ap):
    import concourse.mybir as mybir

    nc = tc.nc
    f32 = mybir.dt.float32
    mult = mybir.AluOpType.mult
    add = mybir.AluOpType.add
    sub = mybir.AluOpType.subtract
    AX = mybir.AxisListType.X
    AF = mybir.ActivationFunctionType

    # DRAM views: [NC_ATOMS, 3] -> [G, 128, 3072]
    c1v = c1_ap.rearrange("(g p a) c -> g p (a c)", g=G, p=P)
    c2v = c2_ap.rearrange("(g p a) c -> g p (a c)", g=G, p=P)

    import contextlib

    with contextlib.ExitStack() as ctx:
        cpool = ctx.enter_context(tc.tile_pool(name="coords", bufs=2))
        spool = ctx.enter_context(tc.tile_pool(name="stats", bufs=1))
        wpool = ctx.enter_context(tc.tile_pool(name="work", bufs=1))
        dpool = ctx.enter_context(tc.tile_pool(name="dummies", bufs=2))
        ppool = ctx.enter_context(tc.tile_pool(name="prod", bufs=3))

        # stats layout: 17 stats x G groups, col = 2*k + g
        #  k 0..8  : S12[i,j] at k=3i+j
        #  k 9..11 : S1_j     (coords1 sums)
        #  k12..14 : S2_i     (coords2 sums)
        #  k15     : SS1, k16: SS2
        ST = spool.tile([P, 34], f32)
        STa = ST[:, :]

        def stcol(k, g):
            return STa[:, 2 * k + g : 2 * k + g + 1]

        # ---------------- per-group statistics ----------------
        # Engine balance: DVE carries 13 fused STT dot products + the tail;
        # ScalarE carries SS/S accumulation + 5 dense product-plane reduces.
        ACT_PAIRS = set()  # moving pairs to ACT serialized ScalarE; keep all on DVE
        first_compute_inst = None
        g1_dma_insts = []
        for g in range(G):
            V1 = cpool.tile([P, W], f32, tag="v1")
            V2 = cpool.tile([P, W], f32, tag="v2")
            d1 = nc.sync.dma_start(out=V1, in_=c1v[g])
            d2 = nc.sync.dma_start(out=V2, in_=c2v[g])
            if g == 1:
                g1_dma_insts = [d1, d2]
            V1a = V1[:, :]
            V2a = V2[:, :]

            dummy_v = dpool.tile([P, 1], f32, tag="dv")
            dummy_s = dpool.tile([P, 1], f32, tag="ds")

            def plane(Va, d):
                return _sv(Va, d, [(3, K)])

            # S12: fused multiply + free-dim reduce on VectorE
            # (scalar_tensor_tensor: out = (in0*1.0)*in1, accum = sum(out);
            #  tensor_tensor_reduce is a custom DVE op that faults on this
            #  runtime, STT is standard ISA and does the same fused job).
            # ACT_PAIRS instead: strided TT product -> dense plane, reduced by
            # a dense ScalarE Copy+accum (cheaper than strided ACT S-copies).
            for i in range(3):
                for j in range(3):
                    if (i, j) in ACT_PAIRS:
                        PRD = ppool.tile([P, K], f32, tag="prd")
                        nc.vector.tensor_tensor(
                            out=PRD[:, :],
                            in0=plane(V2a, i),
                            in1=plane(V1a, j),
                            op=mult,
                        )
                        nc.scalar.activation(
                            out=dummy_s[:, :].broadcast_to((P, K)),
                            in_=PRD[:, :],
                            func=AF.Copy,
                            accum_out=stcol(3 * i + j, g),
                        )
                    else:
                        inst = nc.vector.scalar_tensor_tensor(
                            out=dummy_v[:, :].broadcast_to((P, K)),
                            in0=plane(V2a, i),
                            scalar=1.0,
                            in1=plane(V1a, j),
                            op0=mult,
                            op1=mult,
                            accum_out=stcol(3 * i + j, g),
                        )
                        if first_compute_inst is None:
                            first_compute_inst = inst

            # SS on ScalarE: Square + accumulate over the whole 3072
            nc.scalar.activation(
                out=dummy_s[:, :].broadcast_to((P, W)),
                in_=V1a,
                func=AF.Square,
                accum_out=stcol(15, g),
            )
            nc.scalar.activation(
                out=dummy_s[:, :].broadcast_to((P, W)),
                in_=V2a,
                func=AF.Square,
                accum_out=stcol(16, g),
            )
            # S (per-coord sums) on ScalarE: Copy + accumulate, strided
            for d in range(3):
                nc.scalar.activation(
                    out=dummy_s[:, :].broadcast_to((P, K)),
                    in_=plane(V1a, d),
                    func=AF.Copy,
                    accum_out=stcol(9 + d, g),
                )
                nc.scalar.activation(
                    out=dummy_s[:, :].broadcast_to((P, K)),
                    in_=plane(V2a, d),
                    func=AF.Copy,
                    accum_out=stcol(12 + d, g),
                )

        # Group-1 loads wait for group-0 compute to start so the SDMA engines
        # give group 0 the full HBM bandwidth first (earlier compute start).
        from concourse.tile_rust import add_dep_helper

        for dma_inst in g1_dma_insts:
            add_dep_helper(
                dma_inst.ins, first_compute_inst.ins, sync=True,
                reason="serialize g1 loads behind g0 compute start",
            )

        # ---------------- per-molecule tail ----------------
        def wt(name, w):
            t = wpool.tile([P, w], f32, tag=name)
            return t[:, :]

        T1 = wt("t1", 18)
        COV = wt("cov", 18)
        CCT = wt("cct", 54)
        C18 = wt("c18", 18)
        CS = wt("cs", 18)
        Q = wt("q", 2)
        P2 = wt("p2", 2)
        RP = wt("rp", 2)
        RPW = wt("rpw", 2)
        DT = wt("dt", 36)
        U = wt("u", 36)
        MI = wt("mi", 12)
        W12 = wt("w12", 12)
        DET4 = wt("det4", 4)
        R2 = wt("r2", 2)
        MU = wt("mu", 4)
        MU2 = wt("mu2", 4)
        MU3 = wt("mu3", 4)
        NUM = wt("num", 4)
        DEN = wt("den", 4)
        RD = wt("rd", 4)
        LV4 = wt("lv4", 4)
        MID2 = wt("mid2", 2)
        SG = wt("sg", 2)
        S3 = wt("s3", 2)
        SQ12 = wt("sq12", 12)
        SSUM4 = wt("ssum4", 4)
        V4 = wt("v4", 4)
        VS2 = wt("vs2", 2)
        DIST2 = wt("dist2", 2)

        tsc = nc.vector.tensor_scalar
        stt = nc.vector.scalar_tensor_tensor
        tt = nc.vector.tensor_tensor

        # outer[i,j,g] = S2_i * S1_j
        tt(
            out=_sv(T1, 0, [(6, 3), (2, 3), (1, 2)]),
            in0=_sv(STa, 24, [(2, 3), (0, 3), (1, 2)]),
            in1=_sv(STa, 18, [(0, 3), (2, 3), (1, 2)]),
            op=mult,
        )
        # cov = (S12 - outer*invK) * invK     (per-atom cross covariance)
        tsc(out=T1, in0=T1, scalar1=INVK, scalar2=None, op0=mult)
        nc.vector.tensor_sub(COV, _sv(STa, 0, [(1, 18)]), T1)
        tsc(out=COV, in0=COV, scalar1=INVK, scalar2=None, op0=mult)

        # C = cov^T cov (Gram, symmetric PSD): CCT[a,b,g,i] = cov[i,a,g]*cov[i,b,g]
        # (DVE APs max 3 free dims -> split over a)
        for a in range(3):
            tt(
                out=_sv(CCT, 18 * a, [(6, 3), (3, 2), (1, 3)]),
                in0=_sv(COV, 2 * a, [(0, 3), (1, 2), (6, 3)]),
                in1=_sv(COV, 0, [(2, 3), (1, 2), (6, 3)]),
                op=mult,
            )
        nc.vector.reduce_sum(
            out=_sv(C18, 0, [(2, 9), (1, 2)]),
            in_=_sv(CCT, 0, [(6, 9), (3, 2), (1, 3)]),
            axis=AX,
        )
        # q = tr(C)/3
        nc.vector.reduce_sum(out=Q, in_=_sv(C18, 0, [(1, 2), (8, 3)]), axis=AX)
        tsc(out=Q, in0=Q, scalar1=1.0 / 3.0, scalar2=None, op0=mult)
        # C18 := Cq = C - q I (traceless); diag d_a = C_aa - q
        nc.vector.tensor_sub(
            _sv(C18, 0, [(1, 2), (8, 3)]),
            _sv(C18, 0, [(1, 2), (8, 3)]),
            _sv(Q, 0, [(1, 2), (0, 3)]),
        )
        # p2 = sum_ab Cq_ab^2 ; p = sqrt(p2/6 + eps); rp = 1/p
        tt(out=CS, in0=C18, in1=C18, op=mult)
        nc.vector.reduce_sum(out=P2, in_=_sv(CS, 0, [(1, 2), (2, 9)]), axis=AX)
        tsc(out=P2, in0=P2, scalar1=1.0 / 6.0, scalar2=1e-12, op0=mult, op1=add)
        nc.scalar.activation(out=P2, in_=P2, func=AF.Sqrt)
        nc.vector.reciprocal(out=RP, in_=P2)

        # DT[a,b,m,g]: m=0 -> cov, m=1 -> Cq; batched 3x3 determinant
        nc.vector.tensor_copy(
            _sv(DT, 0, [(12, 3), (4, 3), (1, 2)]), _sv(COV, 0, [(6, 3), (2, 3), (1, 2)])
        )
        nc.vector.tensor_copy(
            _sv(DT, 2, [(12, 3), (4, 3), (1, 2)]), _sv(C18, 0, [(6, 3), (2, 3), (1, 2)])
        )
        # u[x,y,m,g] = DT[1,x]*DT[2,y]  (split over x: DVE APs max 3 free dims)
        for x in range(3):
            tt(
                out=_sv(U, 12 * x, [(4, 3), (2, 2), (1, 2)]),
                in0=_sv(DT, 12 + 4 * x, [(0, 3), (2, 2), (1, 2)]),
                in1=_sv(DT, 24, [(4, 3), (2, 2), (1, 2)]),
                op=mult,
            )
        # minors
        nc.vector.tensor_sub(
            _sv(MI, 0, [(2, 2), (1, 2)]), _sv(U, 20, [(2, 2), (1, 2)]), _sv(U, 28, [(2, 2), (1, 2)])
        )
        nc.vector.tensor_sub(
            _sv(MI, 4, [(2, 2), (1, 2)]), _sv(U, 8, [(2, 2), (1, 2)]), _sv(U, 24, [(2, 2), (1, 2)])
        )
        nc.vector.tensor_sub(
            _sv(MI, 8, [(2, 2), (1, 2)]), _sv(U, 4, [(2, 2), (1, 2)]), _sv(U, 12, [(2, 2), (1, 2)])
        )
        # det = c00*M0 - c01*M1 + c02*M2
        tt(
            out=W12,
            in0=_sv(DT, 0, [(4, 3), (2, 2), (1, 2)]),
            in1=_sv(MI, 0, [(4, 3), (2, 2), (1, 2)]),
            op=mult,
        )
        nc.vector.tensor_sub(DET4, _sv(W12, 0, [(2, 2), (1, 2)]), _sv(W12, 4, [(2, 2), (1, 2)]))
        nc.vector.tensor_add(DET4, DET4, _sv(W12, 8, [(2, 2), (1, 2)]))

        # r = clamp(det(Cq)/(2 p^3), [-1, 1])
        nc.vector.tensor_mul(RPW, RP, RP)
        nc.vector.tensor_mul(RPW, RPW, RP)
        nc.vector.tensor_mul(R2, _sv(DET4, 2, [(1, 2)]), RPW)
        tsc(out=R2, in0=R2, scalar1=0.5, scalar2=1.0, op0=mult, op1=mybir.AluOpType.min)
        tsc(out=R2, in0=R2, scalar1=-1.0, scalar2=None, op0=mybir.AluOpType.max)

        # Solve 4u^3 - 3u = r (roots are cos(acos(r)/3 + 2pi k/3)).
        # Fold to a = |r| in [0, 1]: the largest root u1(a) in [0.866, 1] is
        # always well-separated (gap >= 0.37), so Newton converges fast from a
        # quadratic init; the other two roots come exactly from quadratic
        # deflation u = (-u1 +- sqrt(3 - 3 u1^2))/2 — exact at double roots,
        # where plain Newton is only linearly convergent.
        SGR = wt("sgr", 2)    # sign(r)
        AR = wt("ar", 2)      # |r|
        MUA = wt("mua", 2)    # Newton iterate (largest root for a)
        MSQ = wt("msq", 2)
        DSC = wt("dsc", 2)
        MU6 = wt("mu6", 6)    # (u_max, u_mid, u_min) x g for the original r
        tsc(out=SGR, in0=R2, scalar1=0.0, scalar2=None, op0=mybir.AluOpType.is_lt)
        tsc(out=SGR, in0=SGR, scalar1=-2.0, scalar2=1.0, op0=mult, op1=add)
        nc.vector.tensor_mul(AR, R2, SGR)
        # init: fit of cos(acos(a)/3) at a in {0, .5, 1}; |err| < 3e-3
        nc.vector.tensor_mul(MSQ, AR, AR)
        tsc(out=MSQ, in0=MSQ, scalar1=-0.0268, scalar2=0.8660, op0=mult, op1=add)
        stt(out=MUA, in0=AR, scalar=0.1608, in1=MSQ, op0=mult, op1=add)
        MCU = wt("mcu", 2)
        NU2 = wt("nu2", 2)
        DE2 = wt("de2", 2)
        RD2 = wt("rd2", 2)
        for _ in range(3):
            nc.vector.tensor_mul(MSQ, MUA, MUA)
            nc.vector.tensor_mul(MCU, MSQ, MUA)
            stt(out=NU2, in0=MCU, scalar=8.0, in1=AR, op0=mult, op1=add)
            tsc(out=DE2, in0=MSQ, scalar1=12.0, scalar2=-3.0, op0=mult, op1=add)
            nc.vector.reciprocal(out=RD2, in_=DE2)
            nc.vector.tensor_mul(MUA, NU2, RD2)
        # deflation: disc = sqrt(max(3 - 3 u1^2, 0)); u2 = (disc - u1)/2 (mid),
        # u3 = -(u1 + disc)/2 (smallest)
        nc.vector.tensor_mul(MSQ, MUA, MUA)
        tsc(out=MSQ, in0=MSQ, scalar1=-3.0, scalar2=3.0, op0=mult, op1=add)
        tsc(out=MSQ, in0=MSQ, scalar1=0.0, scalar2=None, op0=mybir.AluOpType.max)
        nc.scalar.activation(out=DSC, in_=MSQ, func=AF.Sqrt)
        # u_max(r) = sgn * (r>=0 ? u1 : u3);  u_min(r) = sgn * (r>=0 ? u3 : u1)
        # with m = (sgn+1)/2: u_max = sgn*(u3 + m*(u1-u3)), u_min = sgn*(u1 - m*(u1-u3))
        MM = wt("mm", 2)      # m
        U3 = wt("u3", 2)
        D13 = wt("d13", 2)
        tsc(out=MM, in0=SGR, scalar1=1.0, scalar2=0.5, op0=add, op1=mult)
        nc.vector.tensor_add(U3, MUA, DSC)
        tsc(out=U3, in0=U3, scalar1=-0.5, scalar2=None, op0=mult)
        nc.vector.tensor_sub(D13, MUA, U3)
        # u_mid(r) = sgn * (disc - u1)/2
        nc.vector.tensor_sub(_sv(MU6, 2, [(1, 2)]), DSC, MUA)
        tsc(out=_sv(MU6, 2, [(1, 2)]), in0=_sv(MU6, 2, [(1, 2)]), scalar1=0.5,
            scalar2=None, op0=mult)
        nc.vector.tensor_mul(_sv(MU6, 2, [(1, 2)]), _sv(MU6, 2, [(1, 2)]), SGR)
        MD = wt("md", 2)
        nc.vector.tensor_mul(MD, MM, D13)
        nc.vector.tensor_add(_sv(MU6, 0, [(1, 2)]), U3, MD)
        nc.vector.tensor_mul(_sv(MU6, 0, [(1, 2)]), _sv(MU6, 0, [(1, 2)]), SGR)
        nc.vector.tensor_sub(_sv(MU6, 4, [(1, 2)]), MUA, MD)
        nc.vector.tensor_mul(_sv(MU6, 4, [(1, 2)]), _sv(MU6, 4, [(1, 2)]), SGR)

        # lambda_k = q + 2 p u_k; sigma = sqrt(lambda); LS6 = (max, mid, min) x g
        LS6 = wt("ls6", 6)
        nc.vector.tensor_mul(LS6, MU6, _sv(P2, 0, [(0, 3), (1, 2)]))
        tsc(out=LS6, in0=LS6, scalar1=2.0, scalar2=None, op0=mult)
        nc.vector.tensor_add(LS6, LS6, _sv(Q, 0, [(0, 3), (1, 2)]))
        tsc(out=LS6, in0=LS6, scalar1=0.0, scalar2=None, op0=mybir.AluOpType.max)
        nc.scalar.activation(out=LS6, in_=LS6, func=AF.Sqrt)

        # sgn = sign(det cov) applied to smallest sigma
        tsc(out=SG, in0=_sv(DET4, 0, [(1, 2)]), scalar1=0.0, scalar2=None,
            op0=mybir.AluOpType.is_lt)
        tsc(out=SG, in0=SG, scalar1=-2.0, scalar2=1.0, op0=mult, op1=add)
        nc.vector.tensor_mul(SG, SG, _sv(LS6, 4, [(1, 2)]))
        nc.vector.tensor_add(S3, _sv(LS6, 0, [(1, 2)]), _sv(LS6, 2, [(1, 2)]))
        nc.vector.tensor_add(S3, S3, SG)

        # var_t = (SS_t - |S_t|^2 * invK) * invK / 3, t in {1, 2}
        nc.vector.tensor_mul(SQ12, _sv(STa, 18, [(1, 12)]), _sv(STa, 18, [(1, 12)]))
        nc.vector.reduce_sum(
            out=SSUM4, in_=_sv(SQ12, 0, [(6, 2), (1, 2), (2, 3)]), axis=AX
        )
        stt(out=V4, in0=SSUM4, scalar=-INVK, in1=_sv(STa, 30, [(1, 4)]), op0=mult, op1=add)
        tsc(out=V4, in0=V4, scalar1=INVK / 3.0, scalar2=None, op0=mult)
        nc.vector.tensor_add(VS2, _sv(V4, 0, [(1, 2)]), _sv(V4, 2, [(1, 2)]))

        # dist = var1 + var2 - (2/3)(s_max + s_mid + sgn*s_min)
        tsc(out=S3, in0=S3, scalar1=-2.0 / 3.0, scalar2=None, op0=mult)
        nc.vector.tensor_add(DIST2, VS2, S3)

        nc.sync.dma_start(out=dist_ap, in_=DIST2)


def _build():
    if "nc" in _CACHE:
        return _CACHE["nc"]
    import concourse.bacc as bacc
    import concourse.tile as tile
    import concourse.mybir as mybir

    nc = bacc.Bacc("TRN2", target_bir_lowering=False, debug=False)
    c1 = nc.dram_tensor("coords1", [NC_ATOMS, 3], mybir.dt.float32, kind="ExternalInput")
    c2 = nc.dram_tensor("coords2", [NC_ATOMS, 3], mybir.dt.float32, kind="ExternalInput")
    dist = nc.dram_tensor("dist", [P, G], mybir.dt.float32, kind="ExternalOutput")
    with tile.TileContext(nc) as tc:
        _build_kernel_body(tc, c1.ap(), c2.ap(), dist.ap())
    nc.compile()
    _CACHE["nc"] = nc
    return nc


# ----------------------------------------------------------------------
# Host glue
# ----------------------------------------------------------------------

def _expected_molecule_ix():
    return (np.arange(N, dtype=np.int64) // K).astype(np.int32)


def _numpy_fallback(coords1, coords2, molecule_ix):
    """Exact mirror of the reference for unexpected input layouts."""
    c1 = np.asarray(coords1, np.float64)
    c2 = np.asarray(coords2, np.float64)
    mol = np.asarray(molecule_ix, np.int64)
    m = M
    counts = np.bincount(mol, minlength=m).astype(np.float64)

    def seg_sum(x):
        out = np.zeros((m,) + x.shape[1:], np.float64)
        np.add.at(out, mol, x)
        return out

    cnt = counts.reshape((m,) + (1,) * 1)
    m1 = seg_sum(c1) / cnt
    m2 = seg_sum(c2) / cnt
    d1 = c1 - m1[mol]
    d2 = c2 - m2[mol]
    outer = d1[:, None, :] * d2[:, :, None]
    cov = seg_sum(outer.reshape(-1, 9)).reshape(m, 3, 3) / counts[:, None, None]
    sigma = np.linalg.svd(cov, compute_uv=False)
    det = np.linalg.det(cov)
    sigma[det < 0, 2] *= -1.0
    sig = sigma.mean(-1)
    var1 = (seg_sum(d1**2) / counts[:, None]).mean(-1)
    var2 = (seg_sum(d2**2) / counts[:, None]).mean(-1)
    dist = var1 + var2 - 2.0 * sig
    return np.float32(dist.mean())


def kernel(coords1, coords2, molecule_ix, _want_results=False):
    coords1 = np.ascontiguousarray(np.asarray(coords1, np.float32))
    coords2 = np.ascontiguousarray(np.asarray(coords2, np.float32))
    molecule_ix = np.asarray(molecule_ix)

    if (
        coords1.shape != (N, 3)
        or molecule_ix.shape != (N,)
        or not np.array_equal(molecule_ix.astype(np.int64), _expected_molecule_ix())
    ):
        return _numpy_fallback(coords1, coords2, molecule_ix)

    from concourse import bass_utils

    nc = _build()
    in_maps = [
        {
            "coords1": np.ascontiguousarray(coords1[c * NC_ATOMS : (c + 1) * NC_ATOMS]),
            "coords2": np.ascontiguousarray(coords2[c * NC_ATOMS : (c + 1) * NC_ATOMS]),
        }
        for c in range(NCORES)
    ]
    res = bass_utils.run_bass_kernel_spmd(nc, in_maps, core_ids=list(range(NCORES)))
    # dist[p, g] = molecule (g*128 + p) of that core's 256
    per_mol = np.concatenate(
        [np.asarray(r["dist"]).T.reshape(-1) for r in res.results]
    )
    out = np.float32(per_mol.mean(dtype=np.float64))
    if _want_results:
        return out, res, per_mol
    return out
